# revision 6
# baseline (speedup 1.0000x reference)
"""Trainium2 Bass kernel for nn_Encoder_78176994721982 (E(n)-GNN encoder), 8 cores SPMD.

Strategy:
  - Edges sorted by destination (row); nodes in 128-node windows; each core owns a
    contiguous range of windows and all edges pointing into it.
  - Node features h replicated in every core's SBUF as a bf16 table
    [128 part = node%128, free = (node//128)*128 features].
  - h[col] per edge: SWDGE dma_gather (SBUF source, transposed output ->
    feature-major), spread over 4 DMA queues (~2 ns/edge).
  - h[row]: never gathered; per-window Q_w = h_win @ W1_row once, then per-subchunk
    one-hot expansion Q_w^T @ M_s on the TensorEngine.
  - segment_sum: one-hot matmul ef2^T @ M_e accumulated in PSUM per window.
  - ELU = max(z,0) + exp(min(z,0)) - 1 via ACT Relu/Exp + DVE max/add; the "-1"s
    are folded into adjusted biases and a degree-correction rank-1 matmul.
  - Node MLP / embedding / head data-parallel on node shards (h state f32).
  - One DRAM AllGather between the two layers re-replicates h.
"""

import numpy as np
import jax
import jax.numpy as jnp

import concourse.bass as bass
import concourse.mybir as mybir
import concourse.tile as tile
from concourse.tile import add_dep_helper
import concourse.bacc as bacc
from concourse.bass_utils import run_bass_kernel_spmd

P = 128
N_CORES = 8
HID = 128
LAT = 64
IN_NODE = 11
NL = 2
TAB_SPLIT = 32768
GQ = 4
GOP = 512

f32 = mybir.dt.float32
bf16 = mybir.dt.bfloat16
i16 = mybir.dt.int16
AF = mybir.ActivationFunctionType
OP = mybir.AluOpType

_compile_cache = {}


def _bf(x):
    return np.asarray(jnp.asarray(np.asarray(x), dtype=jnp.bfloat16))


def _wrap16(idx_vals):
    L = len(idx_vals)
    ops = np.asarray(idx_vals, dtype=np.int16).reshape(L // 16, 16).T
    return np.tile(ops, (8, 1))


class Plan:
    pass


def _host_prep(inputs):
    pl = Plan()
    edges = np.asarray(inputs["edges"])
    row = edges[0].astype(np.int64)
    col = edges[1].astype(np.int64)
    N = int(inputs["n_nodes"])
    NW_real = (N + P - 1) // P
    WPC = (NW_real + N_CORES - 1) // N_CORES
    NW = WPC * N_CORES
    NS = WPC * P
    NTAB = NW * P
    pl.N, pl.NW, pl.WPC, pl.NS, pl.NTAB = N, NW, WPC, NS, NTAB

    pl.split = TAB_SPLIT if NTAB > TAB_SPLIT else NTAB // 2

    x = np.asarray(inputs["x"], dtype=np.float32)
    dd = x[row] - x[col]
    radial = (dd * dd).sum(1)
    attr = np.asarray(inputs["edge_attr"], dtype=np.float32)

    order = np.argsort(row, kind="stable")
    row_s, col_s = row[order], col[order]
    win_of = row_s // P
    lo_mask = col_s < pl.split
    cnt_lo = np.zeros(NW, np.int64)
    cnt_hi = np.zeros(NW, np.int64)
    np.add.at(cnt_lo, win_of[lo_mask], 1)
    np.add.at(cnt_hi, win_of[~lo_mask], 1)
    SL = np.zeros(WPC, np.int64)
    SH = np.zeros(WPC, np.int64)
    for w in range(NW):
        SL[w % WPC] = max(SL[w % WPC], -(-cnt_lo[w] // P))
        SH[w % WPC] = max(SH[w % WPC], -(-cnt_hi[w] // P))
    SL = np.maximum(SL, 1)
    SH = np.maximum(SH, 1)
    pl.SL, pl.SH = SL, SH
    pl.TS = int((SL + SH).sum())
    pl.EP = pl.TS * P

    pl.gops = []
    for wl in range(WPC):
        for half, S in (("lo", int(SL[wl])), ("hi", int(SH[wl]))):
            n = S * P
            off = 0
            while off < n:
                L = min(GOP, n - off)
                pl.gops.append((wl, half, off, L))
                off += L
    pl.idx_cols = sum(L // 16 for (_, _, _, L) in pl.gops)

    start = np.zeros(NW + 1, np.int64)
    np.add.at(start[1:], win_of, 1)
    start = np.cumsum(start)

    seg_all = np.full((N_CORES, P, pl.TS), -1.0, np.float32)
    ra_all = np.zeros((N_CORES, 8, pl.EP), np.float32)
    idx_all = np.zeros((N_CORES, 128, pl.idx_cols), np.int16)
    deg_all = np.zeros((N_CORES, 1, NS), np.float32)
    deg_glob = np.bincount(row_s, minlength=NTAB).astype(np.float32)
    subbase = np.cumsum(np.concatenate([[0], (SL + SH)[:-1]])).astype(np.int64)

    for k in range(N_CORES):
        deg_all[k, 0, :] = deg_glob[k * NS:(k + 1) * NS]
        icol = 0
        for (wl, half, off, L) in pl.gops:
            w = k * WPC + wl
            sel = slice(start[w], start[w + 1])
            cw = col_s[sel]
            rw = row_s[sel]
            m = (cw < pl.split) if half == "lo" else (cw >= pl.split)
            cols_h = cw[m]
            rows_h = rw[m]
            eids_h = order[sel][m]
            base = 0 if half == "lo" else pl.split
            take = np.arange(off, off + L)
            valid = take < len(cols_h)
            idx_vals = np.zeros(L, np.int64)
            idx_vals[valid] = cols_h[take[valid]] - base
            idx_all[k, :, icol:icol + L // 16] = _wrap16(idx_vals)
            tcol0 = int(subbase[wl]) + (0 if half == "lo" else int(SL[wl])) + off // P
            for j in range(L // P):
                tcol = tcol0 + j
                vv = valid[j * P:(j + 1) * P]
                tk = take[j * P:(j + 1) * P]
                segv = np.full(P, -1.0, np.float32)
                segv[vv] = (rows_h[tk[vv]] - w * P).astype(np.float32)
                seg_all[k, :, tcol] = segv
                rr = np.zeros((8, P), np.float32)
                ee = eids_h[tk[vv]]
                rr[0, vv] = radial[ee]
                rr[1:5, vv] = attr[ee].T
                rr[5, vv] = 1.0
                ra_all[k, :, tcol * P:(tcol + 1) * P] = rr
            icol += L // 16

    pl.seg = seg_all.astype(np.float32)
    pl.ra = _bf(ra_all)
    pl.idx = idx_all
    pl.deg = deg_all

    h0 = np.asarray(inputs["h0"], dtype=np.float32)
    h0T = np.zeros((16, NTAB), np.float32)
    h0T[:IN_NODE, :N] = h0.T
    pl.h0T = _bf(h0T)
    pl.h0T_own = np.stack([h0T[:, k * NS:(k + 1) * NS] for k in range(N_CORES)]).astype(np.float32)

    label = np.asarray(inputs["label"], dtype=np.float32)
    lb = np.zeros((8, NTAB), np.float32)
    lb[:7, :N] = label.T
    lb[7] = 1.0
    pl.labelT = np.stack([lb[:, k * NS:(k + 1) * NS] for k in range(N_CORES)])
    eps = np.asarray(inputs["eps"], dtype=np.float32)
    ep = np.zeros((NTAB, LAT), np.float32)
    ep[:N] = eps
    pl.epsT = np.stack([np.ascontiguousarray(ep[k * NS:(k + 1) * NS].T) for k in range(N_CORES)])

    emb_w = np.zeros((16, HID), np.float32)
    emb_w[:IN_NODE] = np.asarray(inputs["emb_w"], np.float32)
    pl.emb_w = _bf(emb_w)
    pl.emb_w32 = emb_w
    pl.emb_b = np.asarray(inputs["emb_b"], np.float32).reshape(HID, 1)
    pl.emb_b_bc = np.tile(np.asarray(inputs["emb_b"], np.float32).reshape(1, HID), (P, 1))

    ew1 = np.asarray(inputs["edge_w1"], np.float32)
    eb1 = np.asarray(inputs["edge_b1"], np.float32)
    ew2 = np.asarray(inputs["edge_w2"], np.float32)
    eb2 = np.asarray(inputs["edge_b2"], np.float32)
    pl.w1r = [_bf(ew1[l, :HID]) for l in range(NL)]
    pl.w1c = [_bf(ew1[l, HID:2 * HID]) for l in range(NL)]
    w1ra = []
    for l in range(NL):
        m = np.zeros((8, HID), np.float32)
        m[0] = ew1[l, 2 * HID]
        m[1:5] = ew1[l, 2 * HID + 1:2 * HID + 5].reshape(4, HID)
        m[5] = eb1[l]
        w1ra.append(_bf(m))
    pl.w1ra = w1ra
    pl.w2 = [_bf(ew2[l]) for l in range(NL)]
    pl.b2adj = [_bf((eb2[l] - ew2[l].sum(0)).reshape(1, HID)) for l in range(NL)]

    nw1 = np.asarray(inputs["node_w1"], np.float32)
    nb1 = np.asarray(inputs["node_b1"], np.float32)
    nw2 = np.asarray(inputs["node_w2"], np.float32)
    nb2 = np.asarray(inputs["node_b2"], np.float32)
    pl.nw1ac = [(nw1[l, :HID] + nw1[l, 2 * HID:]).astype(np.float32) for l in range(NL)]
    pl.nw1b = [_bf(nw1[l, HID:2 * HID]) for l in range(NL)]
    pl.nw1deg = [(-nw1[l, HID:2 * HID].sum(0)).reshape(1, HID).astype(np.float32) for l in range(NL)]
    pl.nb1 = [nb1[l].reshape(HID, 1).astype(np.float32) for l in range(NL)]
    pl.nb1p1 = [(nb1[l] + 1.0).reshape(HID, 1).astype(np.float32) for l in range(NL)]
    pl.nw2 = [nw2[l].astype(np.float32) for l in range(NL)]
    pl.nb2adj = [(nb2[l] - nw2[l].sum(0)).reshape(HID, 1).astype(np.float32) for l in range(NL)]

    muw = np.asarray(inputs["mu_w"], np.float32)
    varw = np.asarray(inputs["var_w"], np.float32)
    pl.muw1 = muw[:HID].astype(np.float32)
    mw2 = np.zeros((8, LAT), np.float32)
    mw2[:7] = muw[HID:]
    mw2[7] = np.asarray(inputs["mu_b"], np.float32)
    pl.muw2 = mw2
    pl.varw1 = varw[:HID].astype(np.float32)
    vw2 = np.zeros((8, LAT), np.float32)
    vw2[:7] = varw[HID:]
    vw2[7] = np.asarray(inputs["var_b"], np.float32)
    pl.varw2 = vw2

    pl.iota = _bf(np.tile(np.arange(P, dtype=np.float32)[None, :], (P, 1)))
    pl.ident = _bf(np.eye(P, dtype=np.float32))
    pl.ident32 = np.eye(P, dtype=np.float32)
    pl.ones_row = _bf(np.ones((1, P), np.float32))
    return pl


def build_nc(pl, debug_taps=False, no_collective=False):
    WPC, TS, NS, NTAB = pl.WPC, pl.TS, pl.NS, pl.NTAB
    SL, SH = pl.SL, pl.SH
    SLmax, SHmax = int(SL.max()), int(SH.max())
    nc = bacc.Bacc("TRN2", target_bir_lowering=False, debug=False,
                   num_devices=N_CORES, num_swdge_queues=GQ)

    def din(name, shape, dt):
        return nc.dram_tensor(name, list(shape), dt, kind="ExternalInput").ap()

    t_idx = din("idx", [128, pl.idx_cols], i16)
    t_seg = din("seg", [P, TS], f32)
    t_ra = din("ra", [8, pl.EP], bf16)
    t_deg = din("deg", [1, NS], f32)
    t_h0T = din("h0T", [16, NTAB], bf16)
    t_h0To = din("h0T_own", [16, NS], f32)
    t_lab = din("labelT", [8, NS], f32)
    t_eps = din("epsT", [LAT, NS], f32)
    t_embw = din("emb_w", [16, HID], bf16)
    t_embw32 = din("emb_w32", [16, HID], f32)
    t_embb = din("emb_b", [HID, 1], f32)
    t_embb_bc = din("emb_b_bc", [P, HID], f32)
    t_w1r = [din(f"w1r{l}", [HID, HID], bf16) for l in range(NL)]
    t_w1c = [din(f"w1c{l}", [HID, HID], bf16) for l in range(NL)]
    t_w1ra = [din(f"w1ra{l}", [8, HID], bf16) for l in range(NL)]
    t_w2 = [din(f"w2{l}", [HID, HID], bf16) for l in range(NL)]
    t_b2adj = [din(f"b2adj{l}", [1, HID], bf16) for l in range(NL)]
    t_nw1ac = [din(f"nw1ac{l}", [HID, HID], f32) for l in range(NL)]
    t_nw1b = [din(f"nw1b{l}", [HID, HID], bf16) for l in range(NL)]
    t_nw1deg = [din(f"nw1deg{l}", [1, HID], f32) for l in range(NL)]
    t_nb1 = [din(f"nb1{l}", [HID, 1], f32) for l in range(NL)]
    t_nb1p1 = [din(f"nb1p1{l}", [HID, 1], f32) for l in range(NL)]
    t_nw2 = [din(f"nw2{l}", [HID, HID], f32) for l in range(NL)]
    t_nb2adj = [din(f"nb2adj{l}", [HID, 1], f32) for l in range(NL)]
    t_muw1 = din("muw1", [HID, LAT], f32)
    t_muw2 = din("muw2", [8, LAT], f32)
    t_varw1 = din("varw1", [HID, LAT], f32)
    t_varw2 = din("varw2", [8, LAT], f32)
    t_iota = din("iota", [P, P], bf16)
    t_ident = din("ident", [P, P], bf16)
    t_ident32 = din("ident32", [P, P], f32)
    t_ones = din("ones_row", [1, P], bf16)
    t_z = nc.dram_tensor("z", [NS, LAT], f32, kind="ExternalOutput").ap()

    if debug_taps:
        t_dbg_tab0 = nc.dram_tensor("dbg_tab0", [P, NTAB], bf16, kind="ExternalOutput").ap()
        t_dbg_agg0 = nc.dram_tensor("dbg_agg0", [P, NS], f32, kind="ExternalOutput").ap()
        t_dbg_h1 = nc.dram_tensor("dbg_h1", [P, NS], f32, kind="ExternalOutput").ap()
    cc_in = nc.dram_tensor("cc_in", [P, NS], bf16).ap()
    cc_out = nc.dram_tensor("cc_out", [N_CORES, P, NS], bf16, addr_space="Shared").ap()

    # node-dimension chunks (256 wide + remainder)
    chunks = []
    off = 0
    while off < NS:
        w = min(256, NS - off)
        chunks.append((off, w))
        off += w

    with tile.TileContext(nc) as tc:
        with tc.tile_pool(name="tabs", bufs=1) as tabs, \
             tc.tile_pool(name="const", bufs=1) as cpool, \
             tc.tile_pool(name="glo", bufs=3) as gpool, \
             tc.tile_pool(name="work", bufs=2) as wp, \
             tc.tile_pool(name="ework", bufs=2) as ew, \
             tc.tile_pool(name="rapool", bufs=1) as rap, \
             tc.tile_pool(name="pmm", bufs=1, space="PSUM") as pmm, \
             tc.tile_pool(name="pt32", bufs=1, space="PSUM") as pt32, \
             tc.tile_pool(name="ptb", bufs=2, space="PSUM") as ptb, \
             tc.tile_pool(name="pz", bufs=2, space="PSUM") as pz, \
             tc.tile_pool(name="pe2", bufs=1, space="PSUM") as pe2, \
             tc.tile_pool(name="pagg", bufs=1, space="PSUM") as pagg:

            tab = tabs.tile([P, NTAB + 16], bf16)
            hT = tabs.tile([P, NS], f32)
            aggT = tabs.tile([P, NS], bf16)
            idx_sb = tabs.tile([128, pl.idx_cols], i16)
            seg_sb = tabs.tile([P, TS], f32)

            _cseq = [0]

            def cload(shape, dt, src):
                _cseq[0] += 1
                t = cpool.tile(shape, dt, tag=f"c{_cseq[0]}")
                nc.sync.dma_start(out=t[:], in_=src[:])
                return t

            c_iota = cload([P, P], bf16, t_iota)
            c_ident = cload([P, P], bf16, t_ident)
            c_ident32 = cload([P, P], f32, t_ident32)
            c_ones = cload([1, P], bf16, t_ones)
            c_embw = cload([16, HID], bf16, t_embw)
            c_embw32 = cload([16, HID], f32, t_embw32)
            c_embb = cload([HID, 1], f32, t_embb)
            c_embb_bc = cload([P, HID], f32, t_embb_bc)
            c_w1r = [cload([HID, HID], bf16, t_w1r[l]) for l in range(NL)]
            c_w1c = [cload([HID, HID], bf16, t_w1c[l]) for l in range(NL)]
            c_w1ra = [cload([8, HID], bf16, t_w1ra[l]) for l in range(NL)]
            c_w2 = [cload([HID, HID], bf16, t_w2[l]) for l in range(NL)]
            c_b2 = [cload([1, HID], bf16, t_b2adj[l]) for l in range(NL)]
            c_nw1ac = [cload([HID, HID], f32, t_nw1ac[l]) for l in range(NL)]
            c_nw1b = [cload([HID, HID], bf16, t_nw1b[l]) for l in range(NL)]
            c_nw1dg = [cload([1, HID], f32, t_nw1deg[l]) for l in range(NL)]
            c_nb1 = [cload([HID, 1], f32, t_nb1[l]) for l in range(NL)]
            c_nb1p1 = [cload([HID, 1], f32, t_nb1p1[l]) for l in range(NL)]
            c_nw2 = [cload([HID, HID], f32, t_nw2[l]) for l in range(NL)]
            c_nb2 = [cload([HID, 1], f32, t_nb2adj[l]) for l in range(NL)]
            c_muw1 = cload([HID, LAT], f32, t_muw1)
            c_muw2 = cload([8, LAT], f32, t_muw2)
            c_varw1 = cload([HID, LAT], f32, t_varw1)
            c_varw2 = cload([8, LAT], f32, t_varw2)

            nc.sync.dma_start(out=idx_sb[:], in_=t_idx[:])
            nc.sync.dma_start(out=seg_sb[:], in_=t_seg[:])

            gsems = [nc.alloc_semaphore(f"gsem{q}") for q in range(GQ)]
            gcount = [0]
            qcount = [0] * GQ

            gop_icol = []
            icol = 0
            for (_, _, _, L) in pl.gops:
                gop_icol.append(icol)
                icol += L // 16

            def issue_gathers(wl, crit=True):
                g_lo = gpool.tile([P, 1, SLmax * P + 16], bf16, tag="glo")
                g_hi = gpool.tile([P, 1, SHmax * P + 16], bf16, tag="ghi")
                gins = []
                import contextlib
                with (tc.tile_critical() if crit else contextlib.nullcontext()):
                    for gi, (wl2, half, off, L) in enumerate(pl.gops):
                        if wl2 != wl:
                            continue
                        g = g_lo if half == "lo" else g_hi
                        src = tab[:, 0:pl.split] if half == "lo" else tab[:, pl.split:NTAB]
                        q = gcount[0] % GQ
                        gg = nc.gpsimd.dma_gather(
                            out_ap=g[:, :, off:off + L],
                            in_ap=src,
                            idxs_ap=idx_sb[:, gop_icol[gi]:gop_icol[gi] + L // 16],
                            num_idxs=L, num_idxs_reg=L, elem_size=HID,
                            transpose=True,
                            sbuf_tokens_per_rank=128,
                            sbuf_free_dim_per_rank=256,
                            queue_num=q,
                        )
                        gg.then_inc(gsems[q], 16)
                        gins.append(gg.ins)
                        gcount[0] += 1
                        qcount[q] += 1
                return g_lo, g_hi, tuple(qcount), gins

            def wait_gathers(g_lo, g_hi, cums, gins, crit=True):
                import contextlib
                with (tc.tile_critical() if crit else contextlib.nullcontext()):
                    wi = None
                    for q in range(GQ):
                        if cums[q]:
                            wi = nc.gpsimd.wait_ge(gsems[q], 16 * cums[q])
                            for gi_ins in gins:
                                add_dep_helper(wi.ins, gi_ins, sync=False,
                                               reason="gather wait ordering")
                    nc.vector.memset(g_lo[:, :, SLmax * P:], 0)
                    nc.vector.memset(g_hi[:, :, SHmax * P:], 0)

            # ---------- embedding: replicated node-major table ----------
            for c in range(NTAB // 512):
                h0t = wp.tile([16, 512], bf16, tag="h0t")
                nc.sync.dma_start(out=h0t[:], in_=t_h0T[:, c * 512:(c + 1) * 512])
                for j in range(4):
                    ps = pmm.tile([P, P], f32, tag="pmm")
                    nc.tensor.matmul(out=ps[:], lhsT=h0t[:, j * P:(j + 1) * P],
                                     rhs=c_embw[:], start=True, stop=True)
                    w = c * 4 + j
                    nc.vector.tensor_tensor(out=tab[:, w * P:(w + 1) * P], in0=ps[:],
                                            in1=c_embb_bc[:], op=OP.add)
            # own-shard hT (f32, feature-major)
            for (coff, cw) in chunks:
                h0o = wp.tile([16, 256], f32, tag="h0o")
                nc.sync.dma_start(out=h0o[:, :cw], in_=t_h0To[:, coff:coff + cw])
                ps = pmm.tile([P, 256], f32, tag="pmm")
                nc.tensor.matmul(out=ps[:, :cw], lhsT=c_embw32[:], rhs=h0o[:, :cw],
                                 start=True, stop=True)
                nc.scalar.activation(out=hT[:, coff:coff + cw], in_=ps[:, :cw],
                                     func=AF.Identity, bias=c_embb[:])

            if debug_taps:
                nc.sync.dma_start(out=t_dbg_tab0[:], in_=tab[:, :NTAB])
            # ---------- message-passing layers ----------
            subbase = np.cumsum(np.concatenate([[0], (SL + SH)[:-1]])).astype(np.int64)
            for l in range(NL):
                pending = [issue_gathers(0)]
                if WPC > 1:
                    pending.append(issue_gathers(1))
                nti = [2]
                for wl in range(WPC):
                    g_lo, g_hi, cums, gins = pending.pop(0)
                    hi_target = wl + 2
                    with tc.tile_critical():
                        wait_gathers(g_lo, g_hi, cums, gins, crit=False)
                        while nti[0] <= min(hi_target, WPC - 1):
                            pending.append(issue_gathers(nti[0], crit=False))
                            nti[0] += 1

                    # per-window: Q_w = h_win @ W1r (h from the local f32 shard)
                    hTw = wp.tile([P, P], bf16, tag="hTw")
                    nc.vector.tensor_copy(out=hTw[:], in_=hT[:, wl * P:(wl + 1) * P])
                    qps = pt32.tile([P, P], f32, tag="pt32")
                    nc.tensor.matmul(out=qps[:], lhsT=hTw[:], rhs=c_w1r[l][:],
                                     start=True, stop=True)
                    qw = wp.tile([P, P], bf16, tag="qw")
                    nc.scalar.copy(out=qw[:], in_=qps[:])
                    Sw = int(SL[wl] + SH[wl])
                    sub_t = int(subbase[wl])
                    ra = rap.tile([8, (SLmax + SHmax) * P], bf16, tag="ra")
                    nc.sync.dma_start(out=ra[:, :Sw * P],
                                      in_=t_ra[:, sub_t * P:(sub_t + Sw) * P])
                    p_agg = pagg.tile([P, P], f32, tag="pagg")

                    for s in range(Sw):
                        half_lo = s < int(SL[wl])
                        goff = s * P if half_lo else (s - int(SL[wl])) * P
                        g = g_lo if half_lo else g_hi
                        tcol = sub_t + s
                        me = ew.tile([P, P], bf16, tag="me")
                        nc.vector.tensor_scalar(
                            out=me[:], in0=c_iota[:],
                            scalar1=seg_sb[:, tcol:tcol + 1],
                            scalar2=None, op0=OP.is_equal)
                        msp = ptb.tile([P, P], bf16, tag="ptb")
                        nc.tensor.transpose(out=msp[:], in_=me[:], identity=c_ident[:])
                        ms = ew.tile([P, P], bf16, tag="ms")
                        nc.scalar.copy(out=ms[:], in_=msp[:])

                        p_z = pz.tile([P, P], f32, tag="pz")
                        nc.tensor.matmul(out=p_z[:], lhsT=qw[:], rhs=ms[:],
                                         start=True, stop=False)
                        nc.tensor.matmul(out=p_z[:], lhsT=c_w1c[l][:],
                                         rhs=g[:, 0, goff:goff + P],
                                         start=False, stop=False)
                        nc.tensor.matmul(out=p_z[:], lhsT=c_w1ra[l][:],
                                         rhs=ra[:, s * P:(s + 1) * P],
                                         start=False, stop=True)
                        # elu(z)+1 == max(z,0) + exp(min(z,0))
                        m1 = ew.tile([P, P], bf16, tag="mm1")
                        nc.vector.tensor_scalar(out=m1[:], in0=p_z[:], scalar1=0.0,
                                                scalar2=None, op0=OP.min)
                        e1 = ew.tile([P, P], bf16, tag="ee")
                        nc.scalar.activation(out=e1[:], in_=m1[:], func=AF.Exp)
                        r1 = ew.tile([P, P], bf16, tag="rr")
                        nc.vector.tensor_scalar(out=r1[:], in0=p_z[:], scalar1=0.0,
                                                scalar2=None, op0=OP.max)
                        ef1 = ew.tile([P, P], bf16, tag="ef")
                        nc.vector.tensor_tensor(out=ef1[:], in0=r1[:], in1=e1[:],
                                                op=OP.add)
                        p_e2 = pe2.tile([P, P], f32, tag="pe2")
                        nc.tensor.matmul(out=p_e2[:], lhsT=ef1[:], rhs=c_w2[l][:],
                                         start=True, stop=False)
                        nc.tensor.matmul(out=p_e2[:], lhsT=c_ones[:], rhs=c_b2[l][:],
                                         start=False, stop=True)
                        t2 = ew.tile([P, P], f32, tag="tt")
                        nc.scalar.activation(out=t2[:], in_=p_e2[:], func=AF.Relu,
                                             scale=-1.0)
                        e2 = ew.tile([P, P], f32, tag="ee")
                        nc.scalar.activation(out=e2[:], in_=t2[:], func=AF.Exp,
                                             scale=-1.0)
                        r2 = ew.tile([P, P], f32, tag="rr")
                        nc.vector.tensor_scalar(out=r2[:], in0=p_e2[:], scalar1=0.0,
                                                scalar2=None, op0=OP.max)
                        ef2 = ew.tile([P, P], bf16, tag="ef")
                        nc.vector.tensor_tensor(out=ef2[:], in0=r2[:], in1=e2[:],
                                                op=OP.add)
                        nc.tensor.matmul(out=p_agg[:], lhsT=ef2[:], rhs=me[:],
                                         start=(s == 0), stop=(s == Sw - 1))
                    nc.vector.tensor_copy(out=aggT[:, wl * P:(wl + 1) * P],
                                          in_=p_agg[:])

                if debug_taps and l == 0:
                    dagg = wp.tile([P, 256], f32, tag="dagg")
                    for (coff, cw) in chunks:
                        nc.vector.tensor_copy(out=dagg[:, :cw], in_=aggT[:, coff:coff + cw])
                        nc.sync.dma_start(out=t_dbg_agg0[:, coff:coff + cw], in_=dagg[:, :cw])
                # ---------- node MLP ----------
                for (coff, cw) in chunks:
                    sl = slice(coff, coff + cw)
                    p_nf = pmm.tile([P, 256], f32, tag="pmm")
                    nc.tensor.matmul(out=p_nf[:, :cw], lhsT=c_nw1ac[l][:],
                                     rhs=hT[:, sl], start=True, stop=False)
                    nc.tensor.matmul(out=p_nf[:, :cw], lhsT=c_nw1b[l][:],
                                     rhs=aggT[:, sl], start=False, stop=False)
                    degc = wp.tile([1, 256], f32, tag="degc")
                    nc.sync.dma_start(out=degc[:, :cw], in_=t_deg[:, sl])
                    nc.tensor.matmul(out=p_nf[:, :cw], lhsT=c_nw1dg[l][:],
                                     rhs=degc[:, :cw], start=False, stop=True)
                    mn = wp.tile([P, 256], f32, tag="tn")
                    nc.vector.tensor_scalar(out=mn[:, :cw], in0=p_nf[:, :cw],
                                            scalar1=c_nb1[l][:], scalar2=0.0,
                                            op0=OP.add, op1=OP.min)
                    en = wp.tile([P, 256], f32, tag="en")
                    nc.scalar.activation(out=en[:, :cw], in_=mn[:, :cw], func=AF.Exp)
                    rn = wp.tile([P, 256], f32, tag="rn")
                    nc.vector.tensor_scalar(out=rn[:, :cw], in0=p_nf[:, :cw],
                                            scalar1=c_nb1[l][:], scalar2=0.0,
                                            op0=OP.add, op1=OP.max)
                    nf1 = wp.tile([P, 256], f32, tag="nf1")
                    nc.vector.tensor_tensor(out=nf1[:, :cw], in0=rn[:, :cw],
                                            in1=en[:, :cw], op=OP.add)
                    p_h = pmm.tile([P, 256], f32, tag="pmm")
                    nc.tensor.matmul(out=p_h[:, :cw], lhsT=c_nw2[l][:],
                                     rhs=nf1[:, :cw], start=True, stop=True)
                    nc.scalar.activation(out=hT[:, sl], in_=p_h[:, :cw],
                                         func=AF.Identity, bias=c_nb2[l][:])

                if debug_taps and l == 0:
                    nc.sync.dma_start(out=t_dbg_h1[:], in_=hT[:])
                # ---------- layer boundary: re-replicate h ----------
                if l == 0:
                    for wl in range(WPC):
                        tp = pt32.tile([P, P], f32, tag="pt32")
                        nc.tensor.transpose(out=tp[:],
                                            in_=hT[:, wl * P:(wl + 1) * P],
                                            identity=c_ident32[:])
                        hb = wp.tile([P, P], bf16, tag="hb")
                        nc.vector.tensor_copy(out=hb[:], in_=tp[:])
                        nc.sync.dma_start(out=cc_in[:, wl * P:(wl + 1) * P],
                                          in_=hb[:])
                    if not no_collective:
                        nc.gpsimd.collective_compute(
                            "AllGather", OP.bypass,
                            replica_groups=[list(range(N_CORES))],
                            ins=[cc_in[:]], outs=[cc_out[:]],
                        )
                    nc.sync.dma_start(
                        out=tab[:, :NTAB].rearrange("p (k c) -> p k c", k=N_CORES),
                        in_=cc_out.rearrange("k p c -> p k c"),
                    )

            # ---------- head ----------
            for wl in range(WPC):
                sl = slice(wl * P, (wl + 1) * P)
                lab = wp.tile([8, P], f32, tag="lab")
                nc.sync.dma_start(out=lab[:], in_=t_lab[:, sl])
                epst = wp.tile([LAT, P], f32, tag="epst")
                nc.sync.dma_start(out=epst[:], in_=t_eps[:, sl])
                p_mu = pz.tile([LAT, P], f32, tag="pz")
                nc.tensor.matmul(out=p_mu[:], lhsT=c_muw1[:], rhs=hT[:, sl],
                                 start=True, stop=False)
                nc.tensor.matmul(out=p_mu[:], lhsT=c_muw2[:], rhs=lab[:],
                                 start=False, stop=True)
                p_lv = pe2.tile([LAT, P], f32, tag="pe2")
                nc.tensor.matmul(out=p_lv[:], lhsT=c_varw1[:], rhs=hT[:, sl],
                                 start=True, stop=False)
                nc.tensor.matmul(out=p_lv[:], lhsT=c_varw2[:], rhs=lab[:],
                                 start=False, stop=True)
                std = wp.tile([LAT, P], f32, tag="std")
                nc.scalar.activation(out=std[:], in_=p_lv[:], func=AF.Exp, scale=0.5)
                m1 = wp.tile([LAT, P], f32, tag="m1")
                nc.vector.tensor_tensor(out=m1[:], in0=epst[:], in1=std[:], op=OP.mult)
                m2 = wp.tile([LAT, P], f32, tag="m2")
                nc.vector.tensor_scalar(out=m2[:], in0=m1[:], scalar1=0.01,
                                        scalar2=None, op0=OP.mult)
                zT = wp.tile([LAT, P], f32, tag="zT")
                nc.vector.tensor_tensor(out=zT[:], in0=p_mu[:], in1=m2[:], op=OP.add)
                ztp = pt32.tile([P, LAT], f32, tag="pt32")
                nc.tensor.transpose(out=ztp[:], in_=zT[:], identity=c_ident32[:LAT, :LAT])
                zsb = wp.tile([P, LAT], f32, tag="zsb")
                nc.vector.tensor_copy(out=zsb[:], in_=ztp[:])
                nc.sync.dma_start(out=t_z[wl * P:(wl + 1) * P, :], in_=zsb[:])
    nc.compile()
    return nc


def _in_maps(pl):
    maps = []
    for k in range(N_CORES):
        m = {
            "idx": pl.idx[k], "seg": pl.seg[k], "ra": pl.ra[k], "deg": pl.deg[k],
            "h0T": pl.h0T, "h0T_own": pl.h0T_own[k], "labelT": pl.labelT[k],
            "epsT": pl.epsT[k],
            "emb_w": pl.emb_w, "emb_w32": pl.emb_w32, "emb_b": pl.emb_b,
            "emb_b_bc": pl.emb_b_bc,
            "muw1": pl.muw1, "muw2": pl.muw2, "varw1": pl.varw1, "varw2": pl.varw2,
            "iota": pl.iota, "ident": pl.ident, "ident32": pl.ident32,
            "ones_row": pl.ones_row,
        }
        for l in range(NL):
            m[f"w1r{l}"] = pl.w1r[l]
            m[f"w1c{l}"] = pl.w1c[l]
            m[f"w1ra{l}"] = pl.w1ra[l]
            m[f"w2{l}"] = pl.w2[l]
            m[f"b2adj{l}"] = pl.b2adj[l]
            m[f"nw1ac{l}"] = pl.nw1ac[l]
            m[f"nw1b{l}"] = pl.nw1b[l]
            m[f"nw1deg{l}"] = pl.nw1deg[l]
            m[f"nb1{l}"] = pl.nb1[l]
            m[f"nb1p1{l}"] = pl.nb1p1[l]
            m[f"nw2{l}"] = pl.nw2[l]
            m[f"nb2adj{l}"] = pl.nb2adj[l]
        maps.append(m)
    return maps


def _prep_and_build(inputs):
    pl = _host_prep(inputs)
    key = (pl.TS, tuple(pl.SL), tuple(pl.SH), pl.idx_cols, pl.NTAB, pl.split)
    if key not in _compile_cache:
        _compile_cache[key] = build_nc(pl)
    return pl, _compile_cache[key]


def kernel(**inputs):
    pl, nc = _prep_and_build(inputs)
    maps = _in_maps(pl)
    res = run_bass_kernel_spmd(nc, maps, list(range(N_CORES)))
    z = np.concatenate([res.results[k]["z"] for k in range(N_CORES)], axis=0)
    return z[:pl.N].astype(np.float32)

